# revision 11
# baseline (speedup 1.0000x reference)
"""Hadamard transform kernel for Trainium2 (8 NeuronCores, SPMD).

Problem: x (8192, 4096) fp32; apply a 128-point Hadamard transform to each
contiguous 128-element group of every row.  Equivalent to
    out = (x.reshape(-1, 128) @ M).reshape(8192, 4096)
where M is the 128x128 butterfly matrix (symmetric, entries +/- 2^-3.5).

bf16 end-to-end (tolerance is 2e-2; bf16 error is ~3e-3):
  - Host pre-scales x by sqrt(2) and casts to bf16; the device matrix is the
    raw +/-1 Hadamard scaled by 2^-4 (exact in bf16), so combined scaling is
    exactly H/sqrt(128).  Halves HBM traffic and quadruples PE throughput
    vs fp32.  Host upcasts the bf16 result back to fp32.

Layout (the host owns both en/decode, so the device sees transposed blocks):
  - Host sends x_dev[c, (t, g, r)] = x[t*128 + r, g*128 + c] per core: the
    within-group element index c on partitions, groups g major in the free
    dim.  Per 512-wide quad (4 groups x 128 rows) ONE matmul with the
    stationary Hadamard matrix computes M @ x^T = (x @ M)^T, i.e. 64
    matmuls of N=512 per core and zero on-chip transposes.
  - PSUM fp32 -> SBUF bf16 evacuation alternates scalar/vector engines;
    the output goes back in the same transposed layout and the host
    permutes it to natural orientation.
  - DMA chunks cover two 128-row tiles -> 16 KiB contiguous per-partition
    lines, 2 MiB per transfer (near peak DMA efficiency); first/last
    chunks are halved to shorten pipeline fill/drain.
"""

import math

import numpy as np
import ml_dtypes

import concourse.bass as bass
import concourse.tile as tile
from concourse import bacc, mybir
from concourse.bass import ts
from concourse.bass_utils import run_bass_kernel_spmd

N_CORES = 8
ROWS, COLS = 8192, 4096
R_CORE = ROWS // N_CORES  # 1024 rows per core
G = 128                   # hadamard group size
NG = COLS // G            # 32 groups per row
NGC = R_CORE * NG         # 32768 groups per core
NT = R_CORE // 128        # 8 row-tiles per core (4096 free elems each)

BF16 = ml_dtypes.bfloat16

# free-dim chunking (in elements of the [128, 32768] device view).
# input: small head chunks so two DMAs queue immediately (single-DMA
# streams ramp slowly) and compute starts early; big middle chunks for
# peak DMA efficiency; small tail for fast drain.  output: finer grain
# so the store stream tracks compute closely instead of lagging 2 MiB.
IN_CHUNKS = [
    (2048, [2048]),
    (2048, [2048]),
    (4096, [4096]),
    (8192, [4096, 4096]),
    (8192, [4096, 4096]),
    (4096, [4096]),
    (2048, [2048]),
    (2048, [2048]),
]
assert sum(c for c, _ in IN_CHUNKS) == NGC
assert all(sum(o) == c for c, o in IN_CHUNKS)


def _hadamard_raw() -> np.ndarray:
    """Raw +/-1 Sylvester Hadamard matrix of order 128 (symmetric)."""
    h = np.array([[1.0]], dtype=np.float64)
    for _ in range(int(math.log2(G))):
        h = np.block([[h, h], [h, -h]])
    return h


def _build_module():
    nc = bacc.Bacc("TRN2", target_bir_lowering=False, debug=False)
    bf16 = mybir.dt.bfloat16
    f32 = mybir.dt.float32
    x_d = nc.dram_tensor("x", [G, NGC], bf16, kind="ExternalInput")
    h_d = nc.dram_tensor("hmat", [G, G], bf16, kind="ExternalInput")
    o_d = nc.dram_tensor("out", [G, NGC], bf16, kind="ExternalOutput")

    with tile.TileContext(nc) as tc:
        with (
            tc.tile_pool(name="const", bufs=1) as cpool,
            tc.tile_pool(name="xin", bufs=4) as xpool,
            tc.tile_pool(name="outb", bufs=4) as opool,
            tc.tile_pool(name="pst", bufs=1, space=bass.MemorySpace.PSUM) as pst,
            tc.tile_pool(name="psm", bufs=6, space=bass.MemorySpace.PSUM) as psm,
        ):
            # PE warmup: dummy transposes with no data deps so the PE's
            # HAM clock-gate opens during the initial DMA wait.  (Padded
            # to a full 2 KiB PSUM bank so no other buf shares the bank.)
            wsb = cpool.tile([G, G], bf16)
            nc.gpsimd.memset(wsb[:], 1.0)
            wp = pst.tile([G, G], bf16, tag="pt", padded_shape=[128, 1024])
            for _ in range(26):
                nc.tensor.transpose(wp[:, :G], wsb[:], wsb[:])

            hm = cpool.tile([G, G], bf16)
            nc.sync.dma_start(hm[:], h_d[:])

            c0 = 0
            qtog = 0
            for cc, out_splits in IN_CHUNKS:
                xt = xpool.tile([128, cc], bf16, tag="xt")
                nc.sync.dma_start(xt[:], x_d[:, c0:c0 + cc])
                x0 = 0
                for oc in out_splits:
                    ot = opool.tile([128, oc], bf16, tag="ot")
                    for qq in range(oc // 512):
                        pm = psm.tile([128, 512], f32)
                        # one matmul per quad: stationary Hadamard, 512
                        # moving columns -> (x @ M)^T for 4 groups at once
                        nc.tensor.matmul(
                            pm[:], hm[:], xt[:, x0 + qq * 512:x0 + (qq + 1) * 512]
                        )
                        if qtog % 2 == 0:
                            nc.scalar.copy(ot[:, ts(qq, 512)], pm[:])
                        else:
                            nc.vector.tensor_copy(ot[:, ts(qq, 512)], pm[:])
                        qtog += 1
                    nc.scalar.dma_start(o_d[:, c0 + x0:c0 + x0 + oc], ot[:])
                    x0 += oc
                c0 += cc

    nc.compile()
    return nc


_NC_CACHE = None


def _get_nc():
    global _NC_CACHE
    if _NC_CACHE is None:
        _NC_CACHE = _build_module()
    return _NC_CACHE


def _in_maps(x: np.ndarray) -> list:
    """Shard, bf16-encode and block-transpose the input for the 8 cores."""
    xs = np.asarray(x, dtype=np.float32) * np.float32(math.sqrt(2.0))
    xb = xs.astype(BF16)
    hmat = (_hadamard_raw() * 0.0625).astype(BF16)
    maps = []
    for c in range(N_CORES):
        shard = xb[c * R_CORE:(c + 1) * R_CORE]          # [1024, 4096]
        dev = shard.reshape(NT, 128, NG, G)              # [t, r, g, c]
        dev = dev.transpose(3, 0, 2, 1).reshape(G, NGC)  # [c, (t, g, r)]
        maps.append({"x": np.ascontiguousarray(dev), "hmat": hmat})
    return maps


def _decode_out(o_dev: np.ndarray) -> np.ndarray:
    """Inverse of the block-transposed layout: [j, (t, g, r)] -> natural."""
    o = o_dev.reshape(G, NT, NG, 128)        # [j, t, g, r]
    return np.ascontiguousarray(
        o.transpose(1, 3, 2, 0).reshape(R_CORE, COLS)
    )


def kernel(x) -> np.ndarray:
    assert x.shape == (ROWS, COLS)
    nc = _get_nc()
    res = run_bass_kernel_spmd(nc, _in_maps(x), core_ids=list(range(N_CORES)))
    out = np.concatenate(
        [_decode_out(r["out"]) for r in res.results], axis=0
    )
    return out.astype(np.float32)
